# revision 13
# baseline (speedup 1.0000x reference)
"""ContentAttention (GNN message passing) Trainium2 Bass kernel.

Math (per reference):
    Wu = h_u @ W.T ; Wi = h_i @ W.T
    a1 = Wu @ a[:d]  (== h_u @ (W.T @ a[:d]))   [N_u]
    a2 = Wi @ a[d:]                              [N_i]
    e = leaky_relu(a1[:,None] + a2[None,:], 0.2)
    e = where(adj, e, -inf);  alpha = softmax(e, -1);  out = alpha @ Wi

Sharding: rows (N_u) split across 8 cores; h_i / W / a replicated.

Per-core device algorithm:
  setup: w1 = W.T@a1v, w2 = W.T@a2v (PE); a1 = h_u @ w1 (PE, via host-side
         transposed h_u); Wi tiles = hiT.T @ W.T (PE, bf16); a2 broadcast
         rows a2b[p,c] = a2[c] via PE matmul with w2-broadcast weights.
  per 128-row tile:
    l = Prelu(a2b + a1) (ScalarE, alpha=0.2); x = Exp(l - 15) (ScalarE)
    p = x * adj, rowsum += p   (one DVE tensor_tensor_reduce, adj int32)
    pT subtiles via PE transpose -> bf16 -> matmul-accumulate out_psum
    alpha = p * (1/rowsum) (DVE 2x tensor_scalar) -> DMA out
    out = out_psum * (1/rowsum) -> DMA out

All softmax math is shift-invariant: exp(l-15) keeps every value < 1 and
well inside the ACT exp spline's accurate range; masked entries are exact 0
so rows renormalize identically to the reference (assumes every row has at
least one neighbor, which holds with overwhelming probability for this
input distribution; verified in test.py for the fixed seed).
"""

import sys

if "/opt/trn_rl_repo" not in sys.path:
    sys.path.insert(0, "/opt/trn_rl_repo")

from contextlib import ExitStack

import numpy as np

import concourse.bacc as bacc
import concourse.tile as tile
from concourse import mybir
from concourse.bass_utils import run_bass_kernel_spmd
from concourse.masks import make_identity

F32 = mybir.dt.float32
BF16 = mybir.dt.bfloat16
I32 = mybir.dt.int32

N_U, N_I, D = 8192, 8192, 128
NCORES = 8
ROWS = N_U // NCORES          # 1024 rows per core
P = 128                       # partitions
RT = ROWS // P                # 8 row tiles per core
NT_I = N_I // P               # 64 item tiles
C = 2048                      # column chunk for elementwise phases
NCH = N_I // C                # 4 chunks per row tile
SUB = C // P                  # 16 [128,128] subtiles per chunk
G = 4                         # subtiles per PSUM transpose group
EXP_SHIFT = -15.0             # exp(l + EXP_SHIFT) keeps p < 1, spline-accurate

import os
KSTAGE = int(os.environ.get("KSTAGE", "4"))   # debug bisect knob
KCORES = int(os.environ.get("KCORES", str(NCORES)))


def _build_body(ctx: ExitStack, tc, adj, huT, hiT, w, wT, a1v, a2v, out, alpha):
    nc = tc.nc
    AF = mybir.ActivationFunctionType
    ALU = mybir.AluOpType

    singles = ctx.enter_context(tc.tile_pool(name="singles", bufs=1))

    # ---------------- setup: params + constants ----------------
    w_sb = singles.tile([P, P], F32)
    nc.sync.dma_start(out=w_sb, in_=w)
    wT_sb = singles.tile([P, P], F32)
    nc.sync.dma_start(out=wT_sb, in_=wT)
    a1v_sb = singles.tile([P, 1], F32)
    nc.sync.dma_start(out=a1v_sb, in_=a1v)
    a2v_sb = singles.tile([P, 1], F32)
    nc.sync.dma_start(out=a2v_sb, in_=a2v)

    ident = singles.tile([P, P], F32)
    make_identity(nc, ident)

    shift_sb = singles.tile([P, 1], F32)   # per-partition exp bias
    nc.vector.memset(shift_sb, EXP_SHIFT)
    # leaky slope as an AP: an immediate alpha adjacent to Exp on the ACT
    # engine crashes the exec unit (NRT_EXEC_UNIT_UNRECOVERABLE 101)
    slope_sb = singles.tile([P, 1], F32)
    nc.vector.memset(slope_sb, 0.2)

    a1_all = singles.tile([P, RT], F32)    # a1 for this core, tile-column t
    a2b = singles.tile([P, N_I], F32)      # a2 broadcast across partitions
    # Wi tiles [c_part, tile, d] + a trailing ones column per tile: the out
    # matmul then also accumulates the softmax denominator in column D.
    wi_sb = singles.tile([P, NT_I, P + 1], BF16)
    nc.vector.memset(wi_sb.rearrange("p a b -> p (a b)"), 1.0)

    with tc.tile_pool(name="setup", bufs=1) as setup_pool, \
         tc.tile_pool(name="setup_ps", bufs=2, space="PSUM") as sps:
        hiT_sb = setup_pool.tile([P, N_I], F32)
        nc.sync.dma_start(out=hiT_sb, in_=hiT)
        huT_sb = setup_pool.tile([P, ROWS], F32)
        nc.sync.dma_start(out=huT_sb, in_=huT)

        # w1 = W.T @ a1v, w2 = W.T @ a2v   (contraction over d = partitions)
        w1_sb = singles.tile([P, 1], F32)
        w2_sb = singles.tile([P, 1], F32)
        ps_w1 = sps.tile([P, 1], F32, tag="psw")
        nc.tensor.matmul(ps_w1, lhsT=w_sb, rhs=a1v_sb)
        nc.vector.tensor_copy(out=w1_sb, in_=ps_w1)
        ps_w2 = sps.tile([P, 1], F32, tag="psw")
        nc.tensor.matmul(ps_w2, lhsT=w_sb, rhs=a2v_sb)
        nc.vector.tensor_copy(out=w2_sb, in_=ps_w2)

        # w2bT[k, j] = w2[k] for all j (broadcast along free dim)
        w2bT = setup_pool.tile([P, P], F32)
        nc.scalar.activation(out=w2bT, in_=ident, func=AF.Identity,
                             bias=w2_sb, scale=0.0)

        # a1[r] = sum_k h_u[r, k] w1[k]  -> a1_all[:, t]
        for t in range(RT):
            ps_a1 = sps.tile([P, 1], F32, tag="psa1")
            nc.tensor.matmul(ps_a1, lhsT=huT_sb[:, t * P:(t + 1) * P], rhs=w1_sb)
            nc.vector.tensor_copy(out=a1_all[:, t:t + 1], in_=ps_a1)

        # a2b tiles: out[j, c] = sum_k w2[k] * h_i[c, k] = a2[c] (all rows equal)
        # weights (w2bT) stay loaded across all 64 matmuls.
        for gti in range(NT_I // G):
            ps_a2 = sps.tile([P, G, P], F32, tag="psa2")
            for i in range(G):
                ti = gti * G + i
                nc.tensor.matmul(ps_a2[:, i, :], lhsT=w2bT,
                                 rhs=hiT_sb[:, ti * P:(ti + 1) * P],
                                 skip_group_check=True)
            nc.vector.tensor_copy(out=a2b[:, gti * G * P:(gti + 1) * G * P],
                                  in_=ps_a2)

        # Wi tiles (bf16): Wi[c, d] = sum_k h_i[c, k] W[d, k]
        hiTb = setup_pool.tile([P, N_I], BF16)
        nc.vector.tensor_copy(out=hiTb, in_=hiT_sb)
        wTb = setup_pool.tile([P, P], BF16)
        nc.vector.tensor_copy(out=wTb, in_=wT_sb)
        for gti in range(NT_I // G):
            ps_wi = sps.tile([P, G, P], F32, tag="pswi")
            for i in range(G):
                ti = gti * G + i
                nc.tensor.matmul(ps_wi[:, i, :],
                                 lhsT=hiTb[:, ti * P:(ti + 1) * P], rhs=wTb,
                                 skip_group_check=True)
            nc.scalar.copy(out=wi_sb[:, gti * G:(gti + 1) * G, 0:P], in_=ps_wi)

    # ---------------- main loop ----------------
    mains = ctx.enter_context(tc.tile_pool(name="mains", bufs=2))
    adjp = ctx.enter_context(tc.tile_pool(name="adjp", bufs=3))
    apool = ctx.enter_context(tc.tile_pool(name="apool", bufs=3))
    spool = ctx.enter_context(tc.tile_pool(name="spool", bufs=8))
    tpool = ctx.enter_context(tc.tile_pool(name="tpool", bufs=3))
    ps_t = ctx.enter_context(tc.tile_pool(name="ps_t", bufs=2, space="PSUM"))
    ps_o = ctx.enter_context(tc.tile_pool(name="ps_o", bufs=2, space="PSUM"))

    for t in range(RT):
        rsl = slice(t * P, (t + 1) * P)
        x_buf = mains.tile([P, N_I], F32, tag="x")
        ps_out = ps_o.tile([P, P + 1], F32, tag="po")
        sub_idx = 0
        for j in range(NCH):
            csl = slice(j * C, (j + 1) * C)
            # l = leaky_relu(a2 + a1, 0.2); x = exp(l - 15)   (both ScalarE)
            nc.scalar.activation(out=x_buf[:, csl], in_=a2b[:, csl],
                                 func=AF.Prelu, bias=a1_all[:, t:t + 1],
                                 scale=1.0, alpha=slope_sb)
            nc.scalar.activation(out=x_buf[:, csl], in_=x_buf[:, csl],
                                 func=AF.Exp, bias=shift_sb, scale=1.0)
            # p = x * adj (int32 0/1 mask)
            adj_t = adjp.tile([P, C], I32, tag="adj")
            nc.sync.dma_start(out=adj_t, in_=adj[rsl, csl])
            nc.vector.tensor_mul(out=x_buf[:, csl], in0=x_buf[:, csl],
                                 in1=adj_t)
            # transpose p subtiles -> bf16 -> accumulate [out | rowsum]
            for g in range(SUB // G):
                ps_tp = ps_t.tile([P, G, P], F32, tag="tp")
                for i in range(G):
                    s0 = j * C + (g * G + i) * P
                    nc.tensor.matmul(ps_tp[:, i, :],
                                     lhsT=x_buf[:, s0:s0 + P],
                                     rhs=ident, is_transpose=True,
                                     skip_group_check=True)
                pT = tpool.tile([P, G, P], BF16, tag="pT")
                if g % 2 == 0:
                    nc.vector.tensor_copy(out=pT, in_=ps_tp)
                else:
                    nc.scalar.copy(out=pT, in_=ps_tp)
                for i in range(G):
                    nc.tensor.matmul(ps_out, lhsT=pT[:, i, :],
                                     rhs=wi_sb[:, sub_idx, :],
                                     start=(sub_idx == 0),
                                     stop=(sub_idx == NT_I - 1),
                                     skip_group_check=True)
                    sub_idx += 1
        # normalize: ps_out[:, D] is the row sum of masked p
        recip = spool.tile([P, 1], F32, tag="recip")
        nc.vector.reciprocal(out=recip, in_=ps_out[:, P:P + 1])
        for j in range(NCH):
            csl = slice(j * C, (j + 1) * C)
            al = apool.tile([P, C], F32, tag="al")
            nc.vector.tensor_scalar_mul(out=al, in0=x_buf[:, csl], scalar1=recip)
            nc.sync.dma_start(out=alpha[rsl, csl], in_=al)
        o_sb = spool.tile([P, P], F32, tag="osb")
        nc.vector.tensor_scalar_mul(out=o_sb, in0=ps_out[:, 0:P], scalar1=recip)
        nc.sync.dma_start(out=out[rsl, :], in_=o_sb)


def build_program():
    nc = bacc.Bacc("TRN2", target_bir_lowering=False, debug=False,
                   num_devices=NCORES)
    adj = nc.dram_tensor("adj", [ROWS, N_I], I32, kind="ExternalInput").ap()
    huT = nc.dram_tensor("huT", [P, ROWS], F32, kind="ExternalInput").ap()
    hiT = nc.dram_tensor("hiT", [P, N_I], F32, kind="ExternalInput").ap()
    w = nc.dram_tensor("w", [P, P], F32, kind="ExternalInput").ap()
    wT = nc.dram_tensor("wT", [P, P], F32, kind="ExternalInput").ap()
    a1v = nc.dram_tensor("a1v", [P, 1], F32, kind="ExternalInput").ap()
    a2v = nc.dram_tensor("a2v", [P, 1], F32, kind="ExternalInput").ap()
    out = nc.dram_tensor("out", [ROWS, D], F32, kind="ExternalOutput").ap()
    alpha = nc.dram_tensor("alpha", [ROWS, N_I], F32, kind="ExternalOutput").ap()

    with tile.TileContext(nc) as tc:
        with ExitStack() as ctx:
            _build_body(ctx, tc, adj, huT, hiT, w, wT, a1v, a2v, out, alpha)
    nc.finalize()
    return nc


def make_in_maps(inputs):
    h_u = np.ascontiguousarray(np.asarray(inputs["h_u"], dtype=np.float32))
    h_i = np.ascontiguousarray(np.asarray(inputs["h_i"], dtype=np.float32))
    ui_adj = np.ascontiguousarray(np.asarray(inputs["ui_adj"], dtype=np.int32))
    W = np.ascontiguousarray(np.asarray(inputs["W"], dtype=np.float32))
    a = np.asarray(inputs["a"], dtype=np.float32)

    huT = np.ascontiguousarray(h_u.T)                      # [128, 8192]
    hiT = np.ascontiguousarray(h_i.T)                      # [128, 8192]
    wT = np.ascontiguousarray(W.T)
    a1v = np.ascontiguousarray(a[:D].reshape(D, 1))
    a2v = np.ascontiguousarray(a[D:].reshape(D, 1))

    in_maps = []
    for m in range(NCORES):
        rs = slice(m * ROWS, (m + 1) * ROWS)
        in_maps.append({
            "adj": np.ascontiguousarray(ui_adj[rs]),
            "huT": np.ascontiguousarray(huT[:, rs]),
            "hiT": hiT,
            "w": W,
            "wT": wT,
            "a1v": a1v,
            "a2v": a2v,
        })
    return in_maps


_NC_CACHE = None


def get_program():
    global _NC_CACHE
    if _NC_CACHE is None:
        _NC_CACHE = build_program()
    return _NC_CACHE


def run(inputs, trace=False, **kwargs):
    nc = get_program()
    in_maps = make_in_maps(inputs)[:KCORES]
    res = run_bass_kernel_spmd(nc, in_maps, core_ids=list(range(KCORES)),
                               trace=trace, **kwargs)
    out = np.concatenate([r["out"] for r in res.results], axis=0)
    alpha = np.concatenate([r["alpha"] for r in res.results], axis=0)
    return (out, alpha), res


def kernel(**inputs):
    (out, alpha), _ = run(inputs, trace=False)
    return out, alpha
